# revision 3
# baseline (speedup 1.0000x reference)
"""Trainium2 Bass kernel for MultiHeadAttention with RoPE + summed relative bias.

Reference computation (B=8, L=512, D=512, H=8, dh=64):
    Q,K,V = x @ W{q,k,v}.T + b ; RoPE(Q,K) (concat variant)
    scores = Q K^T / 8 + rel_bias.sum(-1)   (bias broadcast over batch+heads)
    out = softmax(scores) V @ Wo.T + bo

Sharding: core i <- batch item i (data parallel). The 512MB rel_bias sum is
sharded by query slice: core i reduces rel_bias[0, 64*i:64*(i+1), :, :] over
d, pieces are AllGathered.  The bias slice is streamed in reduced precision
(bf16 by default, optional scaled fp8 with SWDGE cast-DMA) — the 2e-2
tolerance has ~50x margin over bf16 rounding of the bias sum.

Key structure: exp(s + b) = exp(s) * exp(b), so exp(scores) for all heads is
computed while the bias stream is still running; only the elementwise
multiply, ctx matmuls and output projection wait for the AllGather.
Queue plan: stream DMAs alternate sync/scalar (two HWDGE rings, nothing
ahead of them); weights/persistents ride gpsimd (SWDGE).  The first k-chunk
of the stream is emitted before phase 1a so the DVE's program order is
[kc0 reduces | rope | kc1-3 reduces | e_t muls] and the ACT's is
[kc0 shares | transpose copies | exp | kc1-3 shares | eb | phase2 copies].

All internal layouts are "transposed" (contraction dim on partitions):
    xT [d, l], W?T [din, dout], Q'T/K'T [d, l], scoresT/E [lk, lq],
    ctxT [dh(+1), lq].  Softmax normalization is folded into ctxT via an
    appended ones-column in V (rowsum lands on partition 64) and a
    PE-broadcast reciprocal. The 1/sqrt(dh) scale rides the exp's free
    affine (scale=0.125).
"""
import os
import numpy as np

B, L, D, H = 8, 512, 512, 8
DH = D // H          # 64
NCORES = 8
QS = L // NCORES     # 64 q rows per core
NCH = D // 128       # 4 partition chunks

_cached = {}


def _f32(x):
    return np.ascontiguousarray(x, dtype=np.float32)


def _rope_tables():
    # matches reference _apply_rope: freqs = 10000**(-(arange(0,dh,2)/dh))
    freqs = (10000.0 ** (-(np.arange(0, DH, 2, dtype=np.float32) / np.float32(DH)))).astype(np.float32)
    pos = np.arange(L, dtype=np.float32)
    ang = pos[:, None] * freqs[None, :]          # [L, 32] fp32
    cos = np.cos(ang).astype(np.float32)
    sin = np.sin(ang).astype(np.float32)
    return _f32(np.tile(cos, (1, H))), _f32(np.tile(sin, (1, H)))   # [L, 256]


def _rb_dt():
    return os.environ.get("MHA_RB_DT", "bf16")  # bf16 | fp32 | fp8


FP8_SCALE = 16.0


def _build_nc():
    import concourse.bass as bass
    import concourse.mybir as mybir
    import concourse.tile as tile
    from concourse import bacc

    FP = mybir.dt.float32
    BF = mybir.dt.bfloat16
    AF = mybir.ActivationFunctionType
    ALU = mybir.AluOpType
    use_f32r = os.environ.get("MHA_F32R", "0") == "1"
    rb_dt = _rb_dt()
    RBDT = {"bf16": BF, "fp32": FP, "fp8": mybir.dt.float8e4}[rb_dt]
    STDT = BF if rb_dt == "fp8" else RBDT      # stage tile dtype (fp8 casts on DMA)
    act_mod = int(os.environ.get("MHA_ACT_REDUCE_MOD", "3"))  # 0=off; n: every n-th reduce on ACT
    bf16_attn = os.environ.get("MHA_BF16_ATTN", "1") == "1"
    EDT = BF if bf16_attn else FP              # es / eb / e_t / va dtype
    kc0_first = os.environ.get("MHA_KC0_FIRST", "1") == "1"
    # fp8 mode streams on gpsimd (cast-DMA), so weights go to HWDGE instead
    wdma_gps = (os.environ.get("MHA_WDMA_GPS", "1") == "1") and rb_dt != "fp8"
    pdma = lambda: (nc.gpsimd if wdma_gps else nc.sync)

    def mmt(ap):
        return ap.bitcast(mybir.dt.float32r) if use_f32r else ap

    nc = bacc.Bacc(None, target_bir_lowering=False, num_devices=NCORES)

    xT_d = nc.dram_tensor("xT", [D, L], FP, kind="ExternalInput")
    rb_d = nc.dram_tensor("rb", [QS, L, D], RBDT, kind="ExternalInput")
    w_d = {n: nc.dram_tensor(n, [D, D], FP, kind="ExternalInput")
           for n in ("wqT", "wkT", "wvT", "woT")}
    b_d = {n: nc.dram_tensor(n, [1, D], FP, kind="ExternalInput")
           for n in ("bq", "bk", "bv", "bo")}
    cos_d = nc.dram_tensor("cost", [L, 256], FP, kind="ExternalInput")
    sin_d = nc.dram_tensor("sint", [L, 256], FP, kind="ExternalInput")
    id_d = nc.dram_tensor("ident", [128, 128], FP, kind="ExternalInput")
    out_d = nc.dram_tensor("out", [L, D], FP, kind="ExternalOutput")
    piece_ds = [nc.dram_tensor(f"piece{kc}", [128, QS], FP) for kc in range(NCH)]
    gath_ds = [nc.dram_tensor(f"gath{kc}", [NCORES * 128, QS], FP, addr_space="Shared")
               for kc in range(NCH)]

    n_repeat = int(os.environ.get("MHA_REPEAT", "1"))
    NQ = int(os.environ.get("MHA_STAGE_Q", "4"))
    stage_bufs = int(os.environ.get("MHA_STAGE_BUFS", "8"))
    dma_split = os.environ.get("MHA_DMA_SPLIT", "1") == "1"
    skip_stream = os.environ.get("MHA_SKIP_STREAM", "0") == "1"

    with tile.TileContext(nc) as tc:
        with tc.tile_pool(name="persist", bufs=1) as pp, \
             tc.tile_pool(name="stage", bufs=stage_bufs) as sp:

            # ---------- persistent tiles (all on SWDGE so HWDGE rings stay
            # free for the bias stream) ----------
            xt = pp.tile([128, NCH, L], FP)
            pdma().dma_start(out=xt, in_=xT_d.rearrange("(c p) l -> p c l", p=128))
            wo_t = pp.tile([DH, H, D], FP)
            pdma().dma_start(out=wo_t, in_=w_d["woT"].rearrange("(h p) j -> p h j", p=DH))
            cost = pp.tile([128, NCH, 256], FP)
            pdma().dma_start(out=cost, in_=cos_d.rearrange("(c p) k -> p c k", p=128))
            sint = pp.tile([128, NCH, 256], FP)
            pdma().dma_start(out=sint, in_=sin_d.rearrange("(c p) k -> p c k", p=128))
            bt = {}
            for nm in ("bq", "bk", "bv", "bo"):
                t = pp.tile([1, D], FP, tag=f"b_{nm}")
                pdma().dma_start(out=t, in_=b_d[nm][:, :])
                bt[nm] = t
            ident = pp.tile([128, 128], FP)
            pdma().dma_start(out=ident, in_=id_d[:, :])
            ones = pp.tile([128, 128], FP)
            nc.vector.memset(ones, 1.0)

            va = pp.tile([128, NCH, H * (DH + 1)], EDT)     # V + ones col per head
            nc.gpsimd.memset(va, 1.0)
            qt = pp.tile([128, NCH, L], FP)                 # Q'T [d, l]
            kt = pp.tile([128, NCH, L], FP)                 # K'T [d, l]
            eb_k = [pp.tile([128, L], EDT, tag=f"eb{kc}", name=f"eb{kc}")
                    for kc in range(NCH)]                   # exp(biasT) per k-chunk
            pieces_k = [pp.tile([128, QS], FP, tag=f"pieces{kc}", name=f"pieces{kc}")
                        for kc in range(NCH)]               # biasT piece (this core's q)

            def emit_pass():
                red_state = [0]

                def stream_kc(kc):
                    if skip_stream:
                        nc.vector.memset(pieces_k[kc], 0.01)
                        return
                    for qg in range(QS // NQ):
                        st = sp.tile([128, NQ, D], STDT, tag="stage")
                        src = rb_d[qg * NQ:(qg + 1) * NQ, kc * 128:(kc + 1) * 128, :]
                        if rb_dt == "fp8":
                            dma_eng = nc.gpsimd      # SWDGE: cast fp8 -> bf16
                        else:
                            dma_eng = nc.scalar if (dma_split and qg % 2 == 1) else nc.sync
                        dma_eng.dma_start(out=st, in_=src.rearrange("q k d -> k q d"))
                        for qi in range(NQ):
                            col = qg * NQ + qi
                            red_state[0] += 1
                            if act_mod and red_state[0] % act_mod == 0:
                                nc.scalar.activation(
                                    out=st[:, qi, :], in_=st[:, qi, :], func=AF.Copy,
                                    accum_out=pieces_k[kc][:, col:col + 1])
                            else:
                                nc.vector.tensor_scalar(
                                    out=st[:, qi, :], in0=st[:, qi, :],
                                    scalar1=1.0, scalar2=0.0,
                                    op0=ALU.mult, op1=ALU.add,
                                    accum_out=pieces_k[kc][:, col:col + 1])

                # ---------- stream kc0 first: DVE/ACT program order starts
                # with kc0 reduces, DMA rings start moving at t=0 ----------
                if kc0_first:
                    stream_kc(0)

                # ---------- phase 1a: projections + rope + transposes ----------
                with tc.tile_pool(name="wqkv", bufs=1) as wp, \
                     tc.tile_pool(name="rope", bufs=1) as rp, \
                     tc.tile_pool(name="ps_a", bufs=3, space="PSUM") as ps_a, \
                     tc.tile_pool(name="ps_tr", bufs=3, space="PSUM") as ps_tr, \
                     tc.tile_pool(name="tmp", bufs=6) as tp:

                    wts = {}
                    for nm in ("wqT", "wkT", "wvT"):
                        t = wp.tile([128, NCH, D], FP, tag=nm)
                        pdma().dma_start(out=t, in_=w_d[nm].rearrange("(c p) j -> p c j", p=128))
                        wts[nm] = t

                    qp = rp.tile([128, NCH, D], FP, tag="qp")   # roped Q [l, d]
                    kp = rp.tile([128, NCH, D], FP, tag="kp")

                    def proj_chunk(wtile, brow, lc):
                        """psum <- x[lc*128:...,:] @ W.T + b  (chunk of 128 l-rows)"""
                        ps = ps_a.tile([128, 512], FP, tag="proj")
                        for kk in range(NCH):
                            nc.tensor.matmul(
                                ps, lhsT=mmt(xt[:, kk, lc * 128:(lc + 1) * 128]),
                                rhs=mmt(wtile[:, kk, :]),
                                start=(kk == 0), stop=False)
                        nc.tensor.matmul(ps, lhsT=mmt(ones[0:1, 0:128]), rhs=mmt(brow),
                                         start=False, stop=True)
                        return ps

                    def rope(ps, dst, lc):
                        E = ps.rearrange("p (c two) -> p c two", two=2)[:, :, 0]
                        O = ps.rearrange("p (c two) -> p c two", two=2)[:, :, 1]
                        cc = cost[:, lc, :]
                        ss = sint[:, lc, :]
                        t1 = tp.tile([128, 256], FP, tag="t1")
                        t2 = tp.tile([128, 256], FP, tag="t2")
                        nc.vector.tensor_mul(t1, E, cc)
                        nc.vector.tensor_mul(t2, O, ss)
                        dv = dst[:, lc].rearrange("p (h two k) -> p h two k", two=2, k=32)
                        t1r = t1.rearrange("p (h k) -> p h k", k=32)
                        t2r = t2.rearrange("p (h k) -> p h k", k=32)
                        nc.vector.tensor_sub(dv[:, :, 0, :], t1r, t2r)
                        t3 = tp.tile([128, 256], FP, tag="t1")
                        t4 = tp.tile([128, 256], FP, tag="t2")
                        nc.vector.tensor_mul(t3, E, ss)
                        nc.vector.tensor_mul(t4, O, cc)
                        nc.vector.tensor_add(dv[:, :, 1, :], t3.rearrange("p (h k) -> p h k", k=32),
                                             t4.rearrange("p (h k) -> p h k", k=32))

                    for lc in range(NCH):
                        ps = proj_chunk(wts["wqT"], bt["bq"], lc)
                        rope(ps, qp, lc)
                    for lc in range(NCH):
                        ps = proj_chunk(wts["wkT"], bt["bk"], lc)
                        rope(ps, kp, lc)
                    for lc in range(NCH):
                        ps = proj_chunk(wts["wvT"], bt["bv"], lc)
                        dst = va[:, lc].rearrange("p (h c) -> p h c", c=DH + 1)[:, :, 0:DH]
                        nc.scalar.copy(out=dst, in_=ps.rearrange("p (h c) -> p h c", c=DH))

                    # transpose roped Q,K -> [d, l] layout
                    for src, dst in ((qp, qt), (kp, kt)):
                        for lc in range(NCH):
                            for dc in range(NCH):
                                tps = ps_tr.tile([128, 128], FP, tag="tr")
                                nc.tensor.transpose(
                                    tps, in_=src[:, lc, dc * 128:(dc + 1) * 128],
                                    identity=ident)
                                nc.scalar.copy(out=dst[:, dc, lc * 128:(lc + 1) * 128],
                                               in_=tps)

                # ---------- phase 1b: scores + exp for all heads ----------
                with tc.tile_pool(name="es_p", bufs=1) as esp:
                    es = esp.tile([128, H * NCH, L], EDT)    # exp(scoresT/8) per (h, lk-chunk)
                    with tc.tile_pool(name="ps_s", bufs=3, space="PSUM") as ps_s:
                        for h in range(H):
                            dc, po = h // 2, (h % 2) * DH
                            for m in range(NCH):
                                ps = ps_s.tile([128, 512], FP, tag="sc")
                                nc.tensor.matmul(
                                    ps,
                                    lhsT=mmt(kt[po:po + DH, dc, m * 128:(m + 1) * 128]),
                                    rhs=mmt(qt[po:po + DH, dc, :]),
                                    start=True, stop=True)
                                nc.scalar.activation(out=es[:, h * NCH + m, :], in_=ps,
                                                     func=AF.Exp, scale=0.125)

                    # ---------- rest of the bias stream ----------
                    for kc in range((1 if kc0_first else 0), NCH):
                        stream_kc(kc)

                    if os.environ.get("MHA_ONLY_STREAM", "0") == "1":
                        for kc in range(NCH):
                            nc.gpsimd.dma_start(out=piece_ds[kc][:, :], in_=pieces_k[kc][:, :])
                        nc.sync.dma_start(out=out_d[0:128, 0:QS], in_=pieces_k[0][:, :])
                        return

                    # ---------- collectives: allgather bias pieces per k-chunk ----------
                    eb_scale = (1.0 / FP8_SCALE) if rb_dt == "fp8" else 1.0
                    with tc.tile_pool(name="btkp", bufs=2) as btp:
                        for kc in range(NCH):
                            nc.gpsimd.dma_start(out=piece_ds[kc][:, :], in_=pieces_k[kc][:, :])
                            nc.gpsimd.collective_compute(
                                "AllGather", ALU.bypass,
                                replica_groups=[list(range(NCORES))],
                                ins=[piece_ds[kc][:, :]], outs=[gath_ds[kc][:, :]])
                            btk = btp.tile([128, L], FP, tag="btk")
                            nc.gpsimd.dma_start(
                                out=btk.rearrange("p (j q) -> p j q", q=QS),
                                in_=gath_ds[kc].rearrange("(j p) q -> p j q", p=128))
                            nc.scalar.activation(out=eb_k[kc], in_=btk, func=AF.Exp,
                                                 scale=eb_scale)

                    # ---------- phase 2: ctx per head + interleaved out-proj ----------
                    with tc.tile_pool(name="emul", bufs=3) as ep, \
                         tc.tile_pool(name="cu", bufs=3) as cup, \
                         tc.tile_pool(name="nrm1", bufs=1) as nr1, \
                         tc.tile_pool(name="nrm2", bufs=2) as nr2, \
                         tc.tile_pool(name="outp", bufs=2) as op_, \
                         tc.tile_pool(name="ps_ctx", bufs=2, space="PSUM") as pctx, \
                         tc.tile_pool(name="ps_out", bufs=1, space="PSUM") as pout, \
                         tc.tile_pool(name="ps_bc", bufs=2, space="PSUM") as pbc:

                        rs = nr1.tile([DH + 1, 512], FP, tag="rs")
                        ops_tiles = [pout.tile([128, 512], FP, tag=f"ops{m}",
                                               name=f"ops{m}")
                                     for m in range(NCH)]
                        for h in range(H):
                            cps = pctx.tile([DH + 1, 512], FP, tag="ctx")
                            for kc in range(NCH):
                                e_t = ep.tile([128, 512], EDT, tag="e")
                                nc.vector.tensor_mul(e_t, es[:, h * NCH + kc, :],
                                                     eb_k[kc][:, :])
                                nc.tensor.matmul(
                                    cps,
                                    lhsT=va[:, kc, h * (DH + 1):(h + 1) * (DH + 1)],
                                    rhs=e_t,
                                    start=(kc == 0), stop=(kc == NCH - 1))
                            cu = cup.tile([DH + 1, 512], FP, tag="cu")
                            nc.scalar.copy(out=cu, in_=cps)
                            nc.vector.reciprocal(out=rs[DH:DH + 1, :],
                                                 in_=cu[DH:DH + 1, :])
                            nc.vector.tensor_copy(cu[DH:DH + 1, :], rs[DH:DH + 1, :])
                            bps = pbc.tile([DH, 512], FP, tag="bc")
                            nc.tensor.matmul(bps, lhsT=mmt(ones[DH:DH + 1, 0:DH]),
                                             rhs=mmt(cu[DH:DH + 1, :]),
                                             start=True, stop=True)
                            bsb = nr2.tile([DH, 512], FP, tag="bsb")
                            nc.scalar.copy(out=bsb, in_=bps)
                            nc.vector.tensor_mul(cu[0:DH, :], cu[0:DH, :], bsb)
                            cur = cu[0:DH, :]
                            # interleaved output projection accumulation
                            for m in range(NCH):
                                nc.tensor.matmul(
                                    ops_tiles[m],
                                    lhsT=cur[:, m * 128:(m + 1) * 128],
                                    rhs=wo_t[:, h, :],
                                    start=(h == 0), stop=False,
                                    skip_group_check=True)
                        for m in range(NCH):
                            nc.tensor.matmul(ops_tiles[m], lhsT=ones[0:1, 0:128],
                                             rhs=bt["bo"], start=False, stop=True,
                                             skip_group_check=True)
                            osb = op_.tile([128, 512], FP, tag="osb")
                            nc.vector.tensor_copy(osb, ops_tiles[m])
                            nc.sync.dma_start(out=out_d[m * 128:(m + 1) * 128, :], in_=osb)

            for _rep in range(n_repeat):
                emit_pass()
    nc.compile()
    return nc


def _in_maps(x, rel_bias, Wq, bq, Wk, bk, Wv, bv, Wo, bo):
    cost, sint = _rope_tables()
    ident = np.eye(128, dtype=np.float32)
    wqT, wkT, wvT, woT = (_f32(np.asarray(W).T) for W in (Wq, Wk, Wv, Wo))
    x = np.asarray(x)
    rel_bias = np.asarray(rel_bias)
    rb_dt = _rb_dt()
    maps = []
    for c in range(NCORES):
        sl = rel_bias[0, c * QS:(c + 1) * QS]
        if rb_dt == "bf16":
            import ml_dtypes
            rbp = np.ascontiguousarray(sl).astype(ml_dtypes.bfloat16)
        elif rb_dt == "fp8":
            import ml_dtypes
            rbp = np.ascontiguousarray(sl * FP8_SCALE).astype(ml_dtypes.float8_e4m3)
        else:
            rbp = _f32(sl)
        maps.append({
            "xT": _f32(x[c].T),
            "rb": rbp,
            "wqT": wqT, "wkT": wkT, "wvT": wvT, "woT": woT,
            "bq": _f32(np.asarray(bq).reshape(1, D)),
            "bk": _f32(np.asarray(bk).reshape(1, D)),
            "bv": _f32(np.asarray(bv).reshape(1, D)),
            "bo": _f32(np.asarray(bo).reshape(1, D)),
            "cost": cost, "sint": sint,
            "ident": ident,
        })
    return maps


def get_nc():
    if "nc" not in _cached:
        _cached["nc"] = _build_nc()
    return _cached["nc"]


def kernel(x, rel_bias, Wq, bq, Wk, bk, Wv, bv, Wo, bo):
    from concourse.bass_utils import run_bass_kernel_spmd
    nc = get_nc()
    maps = _in_maps(x, rel_bias, Wq, bq, Wk, bk, Wv, bv, Wo, bo)
    res = run_bass_kernel_spmd(nc, maps, core_ids=list(range(NCORES)))
    out = np.stack([res.results[c]["out"] for c in range(NCORES)], axis=0)
    return out.astype(np.float32)
